# revision 9
# baseline (speedup 1.0000x reference)
"""NeuralTPP (GRU + monotone hazard MLP loglik) Bass kernel for 8 trn2 cores.

Problem: B=4096 samples, L=512 steps. Per step t:
  hazard:  pre = tau*w1_tau + h@w1_h.T + b1 ; a = tanh(pre)
           raw = a@w2 + b2 ; phi = softplus(raw)
           dphi = sigmoid(raw) * ((1-a^2)*w1_tau)@w2 ; lam = softplus(dphi)+eps
           tot += sum((log(lam) - phi) * m)
  GRU:     r,z,n gates with scalar input tau; h' = h + m*(1-z)*(n-h)
Output: tot / (sum(mask) + eps)   (scalar f32)

v2 design (bf16, critical-path-optimized). Data parallel: batch 8 x 512,
H-major layout [dim, batch]. The serial recurrence's per-step path is
MM -> sig_r -> rh -> T2 -> tanh -> Bt; everything else off-path:

  - h is never fed to a matmul directly. With zc = m*(1-z) (mask folded
    into the z preact via -30*(1-m)), the update is
      h' = zc*n + (1-zc)*h = Bt - negA,  Bt = zc*n, negA = (zc-1)*h.
    The next step's matmuls take Bt and negA as separate accumulating
    operands (W@h' = W@Bt + (-W)@negA), so h' materialization and the
    matmul wait leave the critical path.
  - One K=3 matmul (MM_X, prefetched) writes all tau/bias parts of the
    gate+hazard preacts (PSUM start), then MM_un/-W and MM_uB/+W
    accumulate the recurrent parts. gbank rows: [zneg|r|h_n|pre].
  - i_n (n-gate input part): per-step K=2 matmul -> inP PSUM.
  - T2 = rh + i_n overwrites gbank rows 64:96 (h_n, already consumed),
    so ONE tanh over gbank[64:128] yields [n; a] in a single ACT.
  - sq = a*a on GpSimd; dots as two K=32 accumulating matmuls (raw into
    dbank rows 0:8, s into rows 32:40 via zero-padded per-(t%4) weight
    variants), evacuated via staging tiles + SBUF DMA every 4 steps.
  - end: batched loglik tail over [128,512] tiles (sigmoid/exp/ln),
    per-partition sums via scalar_tensor_tensor accum_out.
Host: sums the 8 cores' [128,4] partials in f64, divides by mask sum.
"""

import numpy as np

B, L, H, HH = 4096, 512, 32, 32
EPS = 1e-8
BIG = 30.0
NCORES = 8
BC = B // NCORES  # 512 samples per core

_CACHE = {}


def _build_module():
    import concourse.bacc as bacc
    import concourse.mybir as mybir
    import concourse.tile as tile
    from concourse.alu_op_type import AluOpType as ALU

    f32 = mybir.dt.float32
    bf16 = mybir.dt.bfloat16
    AF = mybir.ActivationFunctionType

    nc = bacc.Bacc()

    # xr rows [tau; 1; 1-m; m] x L steps along free dim (bf16)
    xr_d = nc.dram_tensor("xr", [4, L * BC], bf16, kind="ExternalInput")
    # mask rows for the tail, [L, BC] bf16
    mk_d = nc.dram_tensor("mk", [L, BC], bf16, kind="ExternalInput")
    # bf16 weights
    wx_d = nc.dram_tensor("wx", [3, 128], bf16, kind="ExternalInput")
    wu_d = nc.dram_tensor("wu", [32, 128], bf16, kind="ExternalInput")
    wn_d = nc.dram_tensor("wn", [32, 128], bf16, kind="ExternalInput")
    wi_d = nc.dram_tensor("wi", [2, 32], bf16, kind="ExternalInput")
    # dot weight variants: [32, 512]; 16 raw variants [32,16] (w2 at
    # col 16k+k) then 16 s variants (c at col 256+16k+k)
    dw_d = nc.dram_tensor("dw", [32, 512], bf16, kind="ExternalInput")
    # fp32 consts for tail: [c0, b2, eps] per partition
    c0b_d = nc.dram_tensor("c0b", [128, 3], f32, kind="ExternalInput")
    acc_d = nc.dram_tensor("acc_out", [128, 4], f32, kind="ExternalOutput")

    XBLK = 16  # steps per xr DMA block

    with tile.TileContext(nc) as tc:
        with (
            tc.tile_pool(name="consts", bufs=1) as consts,
            tc.tile_pool(name="xrp", bufs=3) as xrp,
            tc.tile_pool(name="hx", bufs=2) as hx_pool,
            tc.tile_pool(name="work", bufs=3) as work,
            tc.tile_pool(name="stg", bufs=2) as stg,
            tc.tile_pool(name="store", bufs=1) as store,
            tc.tile_pool(name="tail", bufs=2) as tailp,
            tc.tile_pool(name="gP", bufs=3, space="PSUM") as gP,
            tc.tile_pool(name="inP", bufs=2, space="PSUM") as inPp,
            tc.tile_pool(name="dP", bufs=2, space="PSUM") as dP,
        ):
            W_x = consts.tile([3, 128], bf16, name="W_x")
            W_u = consts.tile([32, 128], bf16, name="W_u")
            W_un = consts.tile([32, 128], bf16, name="W_un")
            W_i = consts.tile([2, 32], bf16, name="W_i")
            dwt = consts.tile([64, 512], bf16, name="dwt")
            c0b = consts.tile([128, 3], f32)
            nc.sync.dma_start(W_x[:], wx_d[:])
            nc.sync.dma_start(W_u[:], wu_d[:])
            nc.sync.dma_start(W_un[:], wn_d[:])
            nc.sync.dma_start(W_i[:], wi_d[:])
            nc.sync.dma_start(dwt[32:64, :], dw_d[:])
            nc.sync.dma_start(c0b[:], c0b_d[:])
            lhsDr = [dwt[32:64, 16 * k : 16 * k + 16] for k in range(16)]
            lhsDs = [
                dwt[32:64, 256 + 16 * k : 256 + 16 * k + 16] for k in range(16)
            ]

            # stacked raw / s storage: step t -> [t % 128, t // 128, :]
            RAWa = store.tile([128, 4, BC], f32, tag="rawa")
            Sa = store.tile([128, 4, BC], f32, tag="sa")
            ACC = store.tile([128, 4], f32, tag="accs")

            # xr blocks: [4, XBLK*BC] bf16; rows: tau, 1, 1-m, m
            xrt = [None] * (L // XBLK)
            xrt[0] = xrp.tile([4, XBLK * BC], bf16, tag="xr", name="xr0")
            nc.sync.dma_start(xrt[0][:], xr_d[:, 0 : XBLK * BC])

            # h state (bf16); only consumed by negA
            hx = hx_pool.tile([32, BC], bf16, tag="hx")
            nc.vector.memset(hx[:], 0.0)

            gbank_cur = gP.tile([128, BC], f32, tag="gbank")
            nc.tensor.matmul(
                gbank_cur[:], W_x[:], xrt[0][0:3, 0:BC], start=True, stop=True
            )
            inP_cur = inPp.tile([32, BC], f32, tag="inP")
            nc.tensor.matmul(
                inP_cur[:], W_i[:], xrt[0][0:2, 0:BC], start=True, stop=True
            )

            dbank = None

            for t in range(L):
                gb = gbank_cur
                inb = inP_cur
                k = t % 16

                # Emission order IS sync order: a consumer waits for ALL
                # instructions emitted before it on the producer's engine,
                # so path ops are emitted in exact execution order and
                # off-path ops go after the consumers that mustn't wait.

                # ---- critical path ----
                R = work.tile([32, BC], bf16, tag="R")
                nc.scalar.activation(R[:], gb[32:64, :], AF.Sigmoid)
                RH = work.tile([32, BC], bf16, tag="RH")
                nc.vector.tensor_mul(RH[:], R[:], gb[64:96, :])
                ZC = work.tile([32, BC], bf16, tag="ZC")
                nc.scalar.activation(ZC[:], gb[0:32, :], AF.Sigmoid)
                # T2 overwrites h_n rows (already consumed by RH)
                nc.vector.tensor_add(gb[64:96, :], RH[:], inb[:])

                if t < L - 1:
                    if (t + 1) % XBLK == 0:
                        blk = (t + 1) // XBLK
                        xrt[blk] = xrp.tile(
                            [4, XBLK * BC], bf16, tag="xr", name=f"xr{blk}"
                        )
                        nc.sync.dma_start(
                            xrt[blk][:],
                            xr_d[:, XBLK * BC * blk : XBLK * BC * (blk + 1)],
                        )
                    xb_next = xrt[(t + 1) // XBLK]
                    xc = BC * ((t + 1) % XBLK)
                    gbank_next = gP.tile([128, BC], f32, tag="gbank")
                    nc.tensor.matmul(
                        gbank_next[:], W_x[:], xb_next[0:3, xc : xc + BC],
                        start=True, stop=False,
                    )
                    inP_next = inPp.tile([32, BC], f32, tag="inP")
                    nc.tensor.matmul(
                        inP_next[:], W_i[:], xb_next[0:2, xc : xc + BC],
                        start=True, stop=True,
                    )

                NA = work.tile([64, BC], bf16, tag="NA")
                nc.scalar.activation(NA[:], gb[64:128, :], AF.Tanh)

                negA = work.tile([32, BC], bf16, tag="negA")
                nc.vector.scalar_tensor_tensor(
                    negA[:], ZC[:], 1.0, hx[:], op0=ALU.subtract, op1=ALU.mult
                )
                if t < L - 1:
                    nc.tensor.matmul(
                        gbank_next[:], W_un[:], negA[:], start=False, stop=False
                    )

                Bt = work.tile([32, BC], bf16, tag="Bt")
                nc.vector.tensor_mul(Bt[:], ZC[:], NA[0:32, :])
                if t < L - 1:
                    nc.tensor.matmul(
                        gbank_next[:], W_u[:], Bt[:], start=False, stop=True
                    )
                    gbank_cur = gbank_next
                    inP_cur = inP_next

                # ---- off-path ----
                if t < L - 1:
                    hx_next = hx_pool.tile([32, BC], bf16, tag="hx")
                    nc.vector.tensor_sub(hx_next[:], Bt[:], negA[:])
                    hx = hx_next
                SQ = work.tile([64, BC], bf16, tag="SQ")
                nc.gpsimd.tensor_mul(SQ[32:64, :], NA[32:64, :], NA[32:64, :])

                # dots: raw -> dbank row k, s -> dbank row 32+k (16 steps)
                if k == 0:
                    dbank = dP.tile([48, BC], f32, tag="dbank")
                nc.tensor.matmul(
                    dbank[0:16, :], lhsDr[k], NA[32:64, :],
                    start=(k == 0), stop=(k == 15), tile_position=(32, 0),
                )
                nc.tensor.matmul(
                    dbank[32:48, :], lhsDs[k], SQ[32:64, :],
                    start=(k == 0), stop=(k == 15), tile_position=(32, 32),
                )

                if k == 15:
                    g16 = t // 16
                    blk, row = g16 // 8, 16 * (g16 % 8)
                    stR = stg.tile([16, BC], f32, tag="stR", name=f"stR{g16 % 2}")
                    stS = stg.tile([16, BC], f32, tag="stS", name=f"stS{g16 % 2}")
                    nc.scalar.activation(stR[:], dbank[0:16, :], AF.Copy)
                    nc.scalar.activation(stS[:], dbank[32:48, :], AF.Copy)
                    nc.sync.dma_start(RAWa[row : row + 16, blk, :], stR[:])
                    nc.sync.dma_start(Sa[row : row + 16, blk, :], stS[:])

            # ---- batched loglik tail ----
            Mb, SG, ND, PH, SPD, LGL, LL, LLM = ([None] * 4 for _ in range(8))
            for i in range(4):
                Mb[i] = tailp.tile([128, BC], bf16, tag="Mb", name=f"Mb{i}")
                nc.sync.dma_start(Mb[i][:], mk_d[128 * i : 128 * (i + 1), :])
            for i in range(4):
                SG[i] = tailp.tile([128, BC], f32, tag="SG", name=f"SG{i}")
                nc.scalar.activation(
                    SG[i][:], RAWa[:, i, :], AF.Sigmoid, bias=c0b[:, 1:2]
                )
            for i in range(4):
                ND[i] = tailp.tile([128, BC], f32, tag="ND", name=f"ND{i}")
                nc.vector.scalar_tensor_tensor(
                    ND[i][:], Sa[:, i, :], c0b[:, 0:1], SG[i][:],
                    op0=ALU.subtract, op1=ALU.mult,
                )
            # softplus(x) = ln(1 + exp(x)) — no native softplus in this act
            # table set; exp and ln share a table. |raw|, |dphi| < ~8 so no
            # overflow.
            for i in range(4):
                EX = tailp.tile([128, BC], f32, tag="EX", name=f"EX{i}")
                nc.scalar.activation(EX[:], RAWa[:, i, :], AF.Exp, bias=c0b[:, 1:2])
                PH[i] = tailp.tile([128, BC], f32, tag="PH", name=f"PH{i}")
                nc.scalar.activation(PH[i][:], EX[:], AF.Ln, bias=1.0)
                EX2 = tailp.tile([128, BC], f32, tag="EX2", name=f"EX2{i}")
                nc.scalar.activation(EX2[:], ND[i][:], AF.Exp, scale=-1.0)
                SPD[i] = tailp.tile([128, BC], f32, tag="SPD", name=f"SPD{i}")
                nc.scalar.activation(SPD[i][:], EX2[:], AF.Ln, bias=1.0)
            for i in range(4):
                LGL[i] = tailp.tile([128, BC], f32, tag="LGL", name=f"LGL{i}")
                nc.scalar.activation(LGL[i][:], SPD[i][:], AF.Ln, bias=c0b[:, 2:3])
            for i in range(4):
                LL[i] = tailp.tile([128, BC], f32, tag="LL", name=f"LL{i}")
                nc.vector.tensor_sub(LL[i][:], LGL[i][:], PH[i][:])
                LLM[i] = tailp.tile([128, BC], f32, tag="LLM", name=f"LLM{i}")
                nc.vector.scalar_tensor_tensor(
                    LLM[i][:], LL[i][:], 0.0, Mb[i][:],
                    op0=ALU.add, op1=ALU.mult,
                    accum_out=ACC[:, i : i + 1],
                )
            nc.sync.dma_start(acc_d[:], ACC[:])

    nc.finalize()
    return nc


def _prep_host(inputs):
    d = {k: np.asarray(v, np.float32) for k, v in inputs.items()}
    w_ih, w_hh = d["w_ih"], d["w_hh"]
    b_ih, b_hh = d["b_ih"], d["b_hh"]
    w1, b1, w2, b2 = d["w1"], d["b1"], d["w2"], d["b2"]
    w1_tau, w1_h = w1[:, 0], w1[:, 1:]

    # W_x [3,128]: rows tau/1/(1-m) -> col blocks [zneg | r | h_n | pre]
    W_x = np.zeros((3, 128), np.float32)
    W_x[0, 0:32] = -w_ih[32:64, 0]
    W_x[1, 0:32] = -(b_ih[32:64] + b_hh[32:64])
    W_x[2, 0:32] = -BIG
    W_x[0, 32:64] = w_ih[0:32, 0]
    W_x[1, 32:64] = b_ih[0:32] + b_hh[0:32]
    W_x[1, 64:96] = b_hh[64:96]
    W_x[0, 96:128] = w1_tau
    W_x[1, 96:128] = b1

    # W_u [32,128]: recurrent parts (h enters via Bt - negA)
    W_u = np.zeros((32, 128), np.float32)
    W_u[:, 0:32] = -w_hh[32:64, :].T
    W_u[:, 32:64] = w_hh[0:32, :].T
    W_u[:, 64:96] = w_hh[64:96, :].T
    W_u[:, 96:128] = w1_h.T
    W_un = -W_u

    # W_i [2,32]: i_n = w_ih_n * tau + b_ih_n (rows tau, 1)
    W_i = np.stack([w_ih[64:96, 0], b_ih[64:96]], axis=0)

    # dot weight variants [32, 512]
    c = w1_tau * w2
    dw = np.zeros((32, 512), np.float32)
    for k in range(16):
        dw[:, 16 * k + k] = w2
        dw[:, 256 + 16 * k + k] = c

    c0b = np.tile(np.array([[c.sum(), b2[0], EPS]], np.float32), (128, 1))

    def to_bf16(x):
        import jax.numpy as jnp

        return np.asarray(jnp.asarray(x, jnp.bfloat16))

    deltas, mask = d["deltas"], d["mask"]
    wx16, wu16, wn16 = to_bf16(W_x), to_bf16(W_u), to_bf16(W_un)
    wi16, dw16 = to_bf16(W_i), to_bf16(dw)
    in_maps = []
    for i in range(NCORES):
        sl = slice(i * BC, (i + 1) * BC)
        dT = deltas[sl].T  # [L, BC]
        mT = mask[sl].T
        xr = np.empty((4, L * BC), np.float32)
        xr[0] = dT.reshape(-1)
        xr[1] = 1.0
        xr[2] = (1.0 - mT).reshape(-1)
        xr[3] = 0.0
        in_maps.append(
            {
                "xr": to_bf16(xr),
                "mk": to_bf16(mT),
                "wx": wx16,
                "wu": wu16,
                "wn": wn16,
                "wi": wi16,
                "dw": dw16,
                "c0b": c0b,
            }
        )
    return in_maps


def run_on_device(inputs, trace=False):
    from concourse.bass_utils import run_bass_kernel_spmd

    if "nc" not in _CACHE:
        _CACHE["nc"] = _build_module()
    nc = _CACHE["nc"]
    in_maps = _prep_host(inputs)
    res = run_bass_kernel_spmd(nc, in_maps, core_ids=list(range(NCORES)), trace=trace)
    tot = 0.0
    for r in res.results:
        tot += np.asarray(r["acc_out"], np.float64).sum()
    msum = np.asarray(inputs["mask"], np.float64).sum()
    out = np.float32(tot / (msum + EPS))
    return np.asarray(out, np.float32), res


def kernel(**inputs):
    out, _ = run_on_device(inputs, trace=False)
    return out


# revision 22
# speedup vs baseline: 1482.2922x; 1482.2922x over previous
"""NeuralTPP (GRU + monotone hazard MLP loglik) Bass kernel for 8 trn2 cores.

Problem: B=4096 samples, L=512 steps. Per step t:
  hazard:  pre = tau*w1_tau + h@w1_h.T + b1 ; a = tanh(pre)
           raw = a@w2 + b2 ; phi = softplus(raw)
           dphi = sigmoid(raw) * ((1-a^2)*w1_tau)@w2 ; lam = softplus(dphi)+eps
           tot += sum((log(lam) - phi) * m)
  GRU:     r,z,n gates with scalar input tau; h' = h + m*(1-z)*(n-h)
Output: tot / (sum(mask) + eps)   (scalar f32)

v2 design (bf16, critical-path-optimized). Data parallel: batch 8 x 512,
H-major layout [dim, batch]. The serial recurrence's per-step path is
MM -> sig_r -> rh -> T2 -> tanh -> Bt; everything else off-path:

  - h is never fed to a matmul directly. With zc = m*(1-z) (mask folded
    into the z preact via -30*(1-m)), the update is
      h' = zc*n + (1-zc)*h = Bt - negA,  Bt = zc*n, negA = (zc-1)*h.
    The next step's matmuls take Bt and negA as separate accumulating
    operands (W@h' = W@Bt + (-W)@negA), so h' materialization and the
    matmul wait leave the critical path.
  - One K=3 matmul (MM_X, prefetched) writes all tau/bias parts of the
    gate+hazard preacts (PSUM start), then MM_un/-W and MM_uB/+W
    accumulate the recurrent parts. gbank rows: [zneg|r|h_n|pre].
  - i_n (n-gate input part): one K=5 matmul per 4-step group -> inP.
  - T2 = rh + i_n overwrites gbank rows 64:96 (h_n, already consumed),
    so ONE tanh over gbank[64:128] yields [n; a] in a single ACT.
  - sq = a*a and h' = Bt - negA on GpSimd; dots as two K=32 accumulating
    matmuls (raw into dbank rows 0:16, s into rows 32:48 via zero-padded
    per-(t%16) weight variants), one DVE copy + 2 DMAs per 16 steps.
  - every instruction is explicitly paced via tc.tile_set_cur_wait so the
    Tile list-scheduler's static per-engine order matches the intended
    execution order (P=12us/step sim pacing >> real op costs; the HW just
    runs at semaphore speed). The tail is paced after the loop to avoid
    ACT table thrash (sigmoid/tanh vs exp/ln tables).
  - end: batched loglik tail over [128,512] tiles (sigmoid/exp/ln),
    per-partition sums via scalar_tensor_tensor accum_out.
Host: sums the 8 cores' [128,4] partials in f64, divides by mask sum.
"""

import numpy as np

B, L, H, HH = 4096, 512, 32, 32
EPS = 1e-8
BIG = 30.0
NCORES = 8
BC = B // NCORES  # 512 samples per core

_CACHE = {}


def _build_module():
    import concourse.bacc as bacc
    import concourse.mybir as mybir
    import concourse.tile as tile
    from concourse.alu_op_type import AluOpType as ALU

    f32 = mybir.dt.float32
    bf16 = mybir.dt.bfloat16
    AF = mybir.ActivationFunctionType

    nc = bacc.Bacc()

    # xr rows [tau; 1; 1-m; m] x L steps along free dim (bf16)
    xr_d = nc.dram_tensor("xr", [4, L * BC], bf16, kind="ExternalInput")
    # mask rows for the tail, [L, BC] bf16
    mk_d = nc.dram_tensor("mk", [L, BC], bf16, kind="ExternalInput")
    # bf16 weights
    wx_d = nc.dram_tensor("wx", [3, 128], bf16, kind="ExternalInput")
    wu_d = nc.dram_tensor("wu", [32, 128], bf16, kind="ExternalInput")
    wn_d = nc.dram_tensor("wn", [32, 128], bf16, kind="ExternalInput")
    wi_d = nc.dram_tensor("wi", [5, 128], bf16, kind="ExternalInput")
    # tq rows [tau0..tau3; 1] per 4-step group along free dim (bf16)
    tq_d = nc.dram_tensor("tq", [5, L // 4 * BC], bf16, kind="ExternalInput")
    # dot weight variants: [32, 512]; 16 raw variants [32,16] (w2 at
    # col 16k+k) then 16 s variants (c at col 256+16k+k)
    dw_d = nc.dram_tensor("dw", [32, 512], bf16, kind="ExternalInput")
    # fp32 consts for tail: [c0, b2, eps] per partition
    c0b_d = nc.dram_tensor("c0b", [128, 3], f32, kind="ExternalInput")
    acc_d = nc.dram_tensor("acc_out", [128, 4], f32, kind="ExternalOutput")

    XBLK = 16  # steps per xr DMA block

    with tile.TileContext(nc) as tc:
        with (
            tc.tile_pool(name="consts", bufs=1) as consts,
            tc.tile_pool(name="xrp", bufs=3) as xrp,
            tc.tile_pool(name="tqp", bufs=2) as tqp,
            tc.tile_pool(name="hx", bufs=2) as hx_pool,
            tc.tile_pool(name="work", bufs=3) as work,
            tc.tile_pool(name="stg", bufs=2) as stg,
            tc.tile_pool(name="store", bufs=1) as store,
            tc.tile_pool(name="tail", bufs=2) as tailp,
            tc.tile_pool(name="gP", bufs=3, space="PSUM") as gP,
            tc.tile_pool(name="inP", bufs=2, space="PSUM") as inPp,
            tc.tile_pool(name="dP", bufs=2, space="PSUM") as dP,
        ):
            W_x = consts.tile([3, 128], bf16, name="W_x")
            W_u = consts.tile([32, 128], bf16, name="W_u")
            W_un = consts.tile([32, 128], bf16, name="W_un")
            W_i = consts.tile([5, 128], bf16, name="W_i")
            dwt = consts.tile([64, 512], bf16, name="dwt")
            c0b = consts.tile([128, 3], f32)
            nc.sync.dma_start(W_x[:], wx_d[:])
            nc.sync.dma_start(W_u[:], wu_d[:])
            nc.sync.dma_start(W_un[:], wn_d[:])
            nc.sync.dma_start(W_i[:], wi_d[:])
            nc.sync.dma_start(dwt[32:64, :], dw_d[:])
            nc.sync.dma_start(c0b[:], c0b_d[:])
            lhsDr = [dwt[32:64, 16 * k : 16 * k + 16] for k in range(16)]
            lhsDs = [
                dwt[32:64, 256 + 16 * k : 256 + 16 * k + 16] for k in range(16)
            ]

            # stacked raw / s storage: step t -> [t % 128, t // 128, :]
            RAWa = store.tile([128, 4, BC], f32, tag="rawa")
            Sa = store.tile([128, 4, BC], f32, tag="sa")
            ACC = store.tile([128, 4], f32, tag="accs")

            # xr blocks: [4, XBLK*BC] bf16; rows: tau, 1, 1-m, m
            xrt = [None] * (L // XBLK)
            xrt[0] = xrp.tile([4, XBLK * BC], bf16, tag="xr", name="xr0")
            nc.sync.dma_start(xrt[0][:], xr_d[:, 0 : XBLK * BC])
            tqt = [None] * (L // 64)
            tqt[0] = tqp.tile([5, 16 * BC], bf16, tag="tq", name="tq0")
            nc.sync.dma_start(tqt[0][:], tq_d[:, 0 : 16 * BC])

            # h state (bf16); only consumed by negA
            hx = hx_pool.tile([32, BC], bf16, tag="hx")
            nc.vector.memset(hx[:], 0.0)

            gbank_cur = gP.tile([128, BC], f32, tag="gbank")
            nc.tensor.matmul(
                gbank_cur[:], W_x[:], xrt[0][0:3, 0:BC], start=True, stop=True
            )
            inP_cur = inPp.tile([128, BC], f32, tag="inP")
            nc.tensor.matmul(
                inP_cur[:], W_i[:], tqt[0][:, 0:BC], start=True, stop=True
            )

            dbank = None

            # Manual pacing: tile_set_cur_wait pins each instruction's
            # earliest simulated start, which fixes the scheduler's static
            # per-engine order (the HW then runs as fast as semaphores
            # allow, so generous pacing costs nothing at runtime).
            P = 0.012  # ms per step in the pacing model

            for t in range(L):
                gb = gbank_cur
                inb = inP_cur
                k = t % 16
                t0 = t * P

                # ---- critical path ----
                # one sigmoid for both gates: S = [zc; r]
                tc.tile_set_cur_wait(t0)
                S = work.tile([64, BC], bf16, tag="S")
                nc.scalar.activation(S[:], gb[0:64, :], AF.Sigmoid)

                if t < L - 1:
                    tc.tile_set_cur_wait(t0 + 0.0003)
                    if (t + 1) % XBLK == 0:
                        blk = (t + 1) // XBLK
                        xrt[blk] = xrp.tile(
                            [4, XBLK * BC], bf16, tag="xr", name=f"xr{blk}"
                        )
                        nc.sync.dma_start(
                            xrt[blk][:],
                            xr_d[:, XBLK * BC * blk : XBLK * BC * (blk + 1)],
                        )
                    xb_next = xrt[(t + 1) // XBLK]
                    xc = BC * ((t + 1) % XBLK)
                    gbank_next = gP.tile([128, BC], f32, tag="gbank")
                    nc.tensor.matmul(
                        gbank_next[:], W_x[:], xb_next[0:3, xc : xc + BC],
                        start=True, stop=False,
                    )

                # rh at partition 32 so all SBUF operands share a base
                tc.tile_set_cur_wait(t0 + 0.0019)
                RH = work.tile([64, BC], bf16, tag="RH")
                nc.vector.tensor_mul(RH[32:64, :], S[32:64, :], gb[64:96, :])
                # T2 overwrites h_n rows (already consumed by RH)
                tc.tile_set_cur_wait(t0 + 0.0038)
                nc.vector.tensor_add(
                    gb[64:96, :], RH[32:64, :], inb[32 * (t % 4) : 32 * (t % 4) + 32, :]
                )

                # i_n batch for the next 4-step group (prefetched)
                if t % 4 == 0 and t + 4 < L:
                    tc.tile_set_cur_wait(t0 + 0.0040)
                    g_next = t // 4 + 1
                    if g_next % 16 == 0:
                        tb = g_next // 16
                        tqt[tb] = tqp.tile(
                            [5, 16 * BC], bf16, tag="tq", name=f"tq{tb}"
                        )
                        nc.sync.dma_start(
                            tqt[tb][:], tq_d[:, 16 * BC * tb : 16 * BC * (tb + 1)]
                        )
                    tqb = tqt[g_next // 16]
                    tcol = BC * (g_next % 16)
                    inP_next = inPp.tile([128, BC], f32, tag="inP")
                    nc.tensor.matmul(
                        inP_next[:], W_i[:], tqb[:, tcol : tcol + BC],
                        start=True, stop=True,
                    )

                tc.tile_set_cur_wait(t0 + 0.0057)
                NA = work.tile([64, BC], bf16, tag="NA")
                nc.scalar.activation(NA[:], gb[64:128, :], AF.Tanh)

                tc.tile_set_cur_wait(t0 + 0.0060)
                negA = work.tile([32, BC], bf16, tag="negA")
                nc.vector.scalar_tensor_tensor(
                    negA[:], S[0:32, :], 1.0, hx[:], op0=ALU.subtract, op1=ALU.mult
                )
                if t < L - 1:
                    tc.tile_set_cur_wait(t0 + 0.0079)
                    nc.tensor.matmul(
                        gbank_next[:], W_un[:], negA[:], start=False, stop=False
                    )

                tc.tile_set_cur_wait(t0 + 0.0081)
                Bt = work.tile([32, BC], bf16, tag="Bt")
                nc.vector.tensor_mul(Bt[:], S[0:32, :], NA[0:32, :])
                if t < L - 1:
                    tc.tile_set_cur_wait(t0 + 0.0095)
                    nc.tensor.matmul(
                        gbank_next[:], W_u[:], Bt[:], start=False, stop=True
                    )
                    gbank_cur = gbank_next
                    if t % 4 == 3:
                        inP_cur = inP_next

                # ---- off-path ----
                if t < L - 1:
                    tc.tile_set_cur_wait(t0 + 0.0085)
                    hx_next = hx_pool.tile([32, BC], bf16, tag="hx")
                    nc.gpsimd.tensor_sub(hx_next[:], Bt[:], negA[:])
                    hx = hx_next
                tc.tile_set_cur_wait(t0 + 0.0090)
                SQ = work.tile([64, BC], bf16, tag="SQ")
                nc.gpsimd.tensor_mul(SQ[32:64, :], NA[32:64, :], NA[32:64, :])

                # dots: raw -> dbank row k, s -> dbank row 32+k (16 steps)
                tc.tile_set_cur_wait(t0 + 0.0110)
                if k == 0:
                    dbank = dP.tile([48, BC], f32, tag="dbank")
                nc.tensor.matmul(
                    dbank[0:16, :], lhsDr[k], NA[32:64, :],
                    start=(k == 0), stop=(k == 15), tile_position=(32, 0),
                )
                nc.tensor.matmul(
                    dbank[32:48, :], lhsDs[k], SQ[32:64, :],
                    start=(k == 0), stop=(k == 15), tile_position=(32, 32),
                )

                if k == 15:
                    tc.tile_set_cur_wait(t0 + 0.0115)
                    g16 = t // 16
                    blk, row = g16 // 8, 16 * (g16 % 8)
                    stF = stg.tile([48, BC], f32, tag="stF", name=f"stF{g16 % 2}")
                    nc.vector.tensor_copy(stF[:], dbank[:])
                    nc.sync.dma_start(RAWa[row : row + 16, blk, :], stF[0:16, :])
                    nc.sync.dma_start(Sa[row : row + 16, blk, :], stF[32:48, :])

            # tail strictly after the loop in the schedule
            tc.tile_set_cur_wait(L * P + 0.01)

            # ---- batched loglik tail ----
            Mb, SG, ND, PH, SPD, LGL, LL, LLM = ([None] * 4 for _ in range(8))
            for i in range(4):
                Mb[i] = tailp.tile([128, BC], bf16, tag="Mb", name=f"Mb{i}")
                nc.sync.dma_start(Mb[i][:], mk_d[128 * i : 128 * (i + 1), :])
            for i in range(4):
                SG[i] = tailp.tile([128, BC], f32, tag="SG", name=f"SG{i}")
                nc.scalar.activation(
                    SG[i][:], RAWa[:, i, :], AF.Sigmoid, bias=c0b[:, 1:2]
                )
            for i in range(4):
                ND[i] = tailp.tile([128, BC], f32, tag="ND", name=f"ND{i}")
                nc.vector.scalar_tensor_tensor(
                    ND[i][:], Sa[:, i, :], c0b[:, 0:1], SG[i][:],
                    op0=ALU.subtract, op1=ALU.mult,
                )
            # softplus(x) = ln(1 + exp(x)) — no native softplus in this act
            # table set; exp and ln share a table. |raw|, |dphi| < ~8 so no
            # overflow.
            for i in range(4):
                EX = tailp.tile([128, BC], f32, tag="EX", name=f"EX{i}")
                nc.scalar.activation(EX[:], RAWa[:, i, :], AF.Exp, bias=c0b[:, 1:2])
                PH[i] = tailp.tile([128, BC], f32, tag="PH", name=f"PH{i}")
                nc.scalar.activation(PH[i][:], EX[:], AF.Ln, bias=1.0)
                EX2 = tailp.tile([128, BC], f32, tag="EX2", name=f"EX2{i}")
                nc.scalar.activation(EX2[:], ND[i][:], AF.Exp, scale=-1.0)
                SPD[i] = tailp.tile([128, BC], f32, tag="SPD", name=f"SPD{i}")
                nc.scalar.activation(SPD[i][:], EX2[:], AF.Ln, bias=1.0)
            for i in range(4):
                LGL[i] = tailp.tile([128, BC], f32, tag="LGL", name=f"LGL{i}")
                nc.scalar.activation(LGL[i][:], SPD[i][:], AF.Ln, bias=c0b[:, 2:3])
            for i in range(4):
                LL[i] = tailp.tile([128, BC], f32, tag="LL", name=f"LL{i}")
                nc.vector.tensor_sub(LL[i][:], LGL[i][:], PH[i][:])
                LLM[i] = tailp.tile([128, BC], f32, tag="LLM", name=f"LLM{i}")
                nc.vector.scalar_tensor_tensor(
                    LLM[i][:], LL[i][:], 0.0, Mb[i][:],
                    op0=ALU.add, op1=ALU.mult,
                    accum_out=ACC[:, i : i + 1],
                )
            nc.sync.dma_start(acc_d[:], ACC[:])

    nc.finalize()
    return nc


def _prep_host(inputs):
    d = {k: np.asarray(v, np.float32) for k, v in inputs.items()}
    w_ih, w_hh = d["w_ih"], d["w_hh"]
    b_ih, b_hh = d["b_ih"], d["b_hh"]
    w1, b1, w2, b2 = d["w1"], d["b1"], d["w2"], d["b2"]
    w1_tau, w1_h = w1[:, 0], w1[:, 1:]

    # W_x [3,128]: rows tau/1/(1-m) -> col blocks [zneg | r | h_n | pre]
    W_x = np.zeros((3, 128), np.float32)
    W_x[0, 0:32] = -w_ih[32:64, 0]
    W_x[1, 0:32] = -(b_ih[32:64] + b_hh[32:64])
    W_x[2, 0:32] = -BIG
    W_x[0, 32:64] = w_ih[0:32, 0]
    W_x[1, 32:64] = b_ih[0:32] + b_hh[0:32]
    W_x[1, 64:96] = b_hh[64:96]
    W_x[0, 96:128] = w1_tau
    W_x[1, 96:128] = b1

    # W_u [32,128]: recurrent parts (h enters via Bt - negA)
    W_u = np.zeros((32, 128), np.float32)
    W_u[:, 0:32] = -w_hh[32:64, :].T
    W_u[:, 32:64] = w_hh[0:32, :].T
    W_u[:, 64:96] = w_hh[64:96, :].T
    W_u[:, 96:128] = w1_h.T
    W_un = -W_u

    # W_i [5,128]: i_n for 4 steps; col block 32g+j: row g = w_ih_n[j],
    # row 4 = b_ih_n[j]
    W_i = np.zeros((5, 128), np.float32)
    for g in range(4):
        W_i[g, 32 * g : 32 * g + 32] = w_ih[64:96, 0]
        W_i[4, 32 * g : 32 * g + 32] = b_ih[64:96]

    # dot weight variants [32, 512]
    c = w1_tau * w2
    dw = np.zeros((32, 512), np.float32)
    for k in range(16):
        dw[:, 16 * k + k] = w2
        dw[:, 256 + 16 * k + k] = c

    c0b = np.tile(np.array([[c.sum(), b2[0], EPS]], np.float32), (128, 1))

    def to_bf16(x):
        import jax.numpy as jnp

        return np.asarray(jnp.asarray(x, jnp.bfloat16))

    deltas, mask = d["deltas"], d["mask"]
    wx16, wu16, wn16 = to_bf16(W_x), to_bf16(W_u), to_bf16(W_un)
    wi16, dw16 = to_bf16(W_i), to_bf16(dw)
    in_maps = []
    for i in range(NCORES):
        sl = slice(i * BC, (i + 1) * BC)
        dT = deltas[sl].T  # [L, BC]
        mT = mask[sl].T
        xr = np.empty((4, L * BC), np.float32)
        xr[0] = dT.reshape(-1)
        xr[1] = 1.0
        xr[2] = (1.0 - mT).reshape(-1)
        xr[3] = 0.0
        tq = np.empty((5, L // 4 * BC), np.float32)
        tq[0:4] = dT.reshape(L // 4, 4, BC).transpose(1, 0, 2).reshape(4, -1)
        tq[4] = 1.0
        in_maps.append(
            {
                "xr": to_bf16(xr),
                "tq": to_bf16(tq),
                "mk": to_bf16(mT),
                "wx": wx16,
                "wu": wu16,
                "wn": wn16,
                "wi": wi16,
                "dw": dw16,
                "c0b": c0b,
            }
        )
    return in_maps


def run_on_device(inputs, trace=False):
    from concourse.bass_utils import run_bass_kernel_spmd

    if "nc" not in _CACHE:
        _CACHE["nc"] = _build_module()
    nc = _CACHE["nc"]
    in_maps = _prep_host(inputs)
    res = run_bass_kernel_spmd(nc, in_maps, core_ids=list(range(NCORES)), trace=trace)
    tot = 0.0
    for r in res.results:
        tot += np.asarray(r["acc_out"], np.float64).sum()
    msum = np.asarray(inputs["mask"], np.float64).sum()
    out = np.float32(tot / (msum + EPS))
    return np.asarray(out, np.float32), res


def kernel(**inputs):
    out, _ = run_on_device(inputs, trace=False)
    return out


# revision 25
# speedup vs baseline: 1515.1582x; 1.0222x over previous
"""NeuralTPP (GRU + monotone hazard MLP loglik) Bass kernel for 8 trn2 cores.

Problem: B=4096 samples, L=512 steps. Per step t:
  hazard:  pre = tau*w1_tau + h@w1_h.T + b1 ; a = tanh(pre)
           raw = a@w2 + b2 ; phi = softplus(raw)
           dphi = sigmoid(raw) * ((1-a^2)*w1_tau)@w2 ; lam = softplus(dphi)+eps
           tot += sum((log(lam) - phi) * m)
  GRU:     r,z,n gates with scalar input tau; h' = h + m*(1-z)*(n-h)
Output: tot / (sum(mask) + eps)   (scalar f32)

v2 design (bf16, critical-path-optimized). Data parallel: batch 8 x 512,
H-major layout [dim, batch]. The serial recurrence's per-step path is
MM -> sig_r -> rh -> T2 -> tanh -> Bt; everything else off-path:

  - h is never fed to a matmul directly. With zc = m*(1-z) (mask folded
    into the z preact via -30*(1-m)), the update is
      h' = zc*n + (1-zc)*h = Bt - negA,  Bt = zc*n, negA = (zc-1)*h.
    The next step's matmuls take Bt and negA as separate accumulating
    operands (W@h' = W@Bt + (-W)@negA), so h' materialization and the
    matmul wait leave the critical path.
  - One K=3 matmul (MM_X, prefetched) writes all tau/bias parts of the
    gate+hazard preacts (PSUM start), then MM_un/-W and MM_uB/+W
    accumulate the recurrent parts. gbank rows: [zneg|r|h_n|pre].
  - i_n (n-gate input part): one K=5 matmul per 4-step group -> inP.
  - T2 = rh + i_n overwrites gbank rows 64:96 (h_n, already consumed),
    so ONE tanh over gbank[64:128] yields [n; a] in a single ACT.
  - sq = a*a and h' = Bt - negA on GpSimd; dots as two K=32 accumulating
    matmuls (raw into dbank rows 0:16, s into rows 32:48 via zero-padded
    per-(t%16) weight variants), one DVE copy + 2 DMAs per 16 steps.
  - every instruction is explicitly paced via tc.tile_set_cur_wait so the
    Tile list-scheduler's static per-engine order matches the intended
    execution order (P=12us/step sim pacing >> real op costs; the HW just
    runs at semaphore speed). The tail is paced after the loop to avoid
    ACT table thrash (sigmoid/tanh vs exp/ln tables).
  - end: batched loglik tail over [128,512] tiles (sigmoid/exp/ln),
    per-partition sums via scalar_tensor_tensor accum_out.
Host: sums the 8 cores' [128,4] partials in f64, divides by mask sum.
"""

import numpy as np

B, L, H, HH = 4096, 512, 32, 32
EPS = 1e-8
BIG = 30.0
NCORES = 8
BC = B // NCORES  # 512 samples per core

_CACHE = {}


def _build_module():
    import concourse.bacc as bacc
    import concourse.mybir as mybir
    import concourse.tile as tile
    from concourse.alu_op_type import AluOpType as ALU

    f32 = mybir.dt.float32
    bf16 = mybir.dt.bfloat16
    AF = mybir.ActivationFunctionType

    nc = bacc.Bacc()

    # xr rows [tau; 1; 1-m; m] x L steps along free dim (bf16)
    xr_d = nc.dram_tensor("xr", [4, L * BC], bf16, kind="ExternalInput")
    # mask rows for the tail, [L, BC] bf16
    mk_d = nc.dram_tensor("mk", [L, BC], bf16, kind="ExternalInput")
    # bf16 weights
    wx_d = nc.dram_tensor("wx", [3, 128], bf16, kind="ExternalInput")
    wu_d = nc.dram_tensor("wu", [32, 128], bf16, kind="ExternalInput")
    wn_d = nc.dram_tensor("wn", [32, 128], bf16, kind="ExternalInput")
    wi_d = nc.dram_tensor("wi", [5, 128], bf16, kind="ExternalInput")
    # tq rows [tau0..tau3; 1] per 4-step group along free dim (bf16)
    tq_d = nc.dram_tensor("tq", [5, L // 4 * BC], bf16, kind="ExternalInput")
    # dot weight variants: [32, 512]; 16 raw variants [32,16] (w2 at
    # col 16k+k) then 16 s variants (c at col 256+16k+k)
    dw_d = nc.dram_tensor("dw", [32, 512], bf16, kind="ExternalInput")
    # fp32 consts for tail: [c0, b2, eps] per partition
    c0b_d = nc.dram_tensor("c0b", [128, 3], f32, kind="ExternalInput")
    acc_d = nc.dram_tensor("acc_out", [128, 4], f32, kind="ExternalOutput")

    XBLK = 16  # steps per xr DMA block

    with tile.TileContext(nc) as tc:
        with (
            tc.tile_pool(name="consts", bufs=1) as consts,
            tc.tile_pool(name="xrp", bufs=3) as xrp,
            tc.tile_pool(name="tqp", bufs=2) as tqp,
            tc.tile_pool(name="hx", bufs=2) as hx_pool,
            tc.tile_pool(name="work", bufs=3) as work,
            tc.tile_pool(name="stg", bufs=2) as stg,
            tc.tile_pool(name="store", bufs=1) as store,
            tc.tile_pool(name="tail", bufs=2) as tailp,
            tc.tile_pool(name="gP", bufs=3, space="PSUM") as gP,
            tc.tile_pool(name="inP", bufs=2, space="PSUM") as inPp,
            tc.tile_pool(name="dP", bufs=2, space="PSUM") as dP,
        ):
            W_x = consts.tile([3, 128], bf16, name="W_x")
            W_u = consts.tile([32, 128], bf16, name="W_u")
            W_un = consts.tile([32, 128], bf16, name="W_un")
            W_i = consts.tile([5, 128], bf16, name="W_i")
            dwt = consts.tile([64, 512], bf16, name="dwt")
            c0b = consts.tile([128, 3], f32)
            nc.sync.dma_start(W_x[:], wx_d[:])
            nc.sync.dma_start(W_u[:], wu_d[:])
            nc.sync.dma_start(W_un[:], wn_d[:])
            nc.sync.dma_start(W_i[:], wi_d[:])
            nc.sync.dma_start(dwt[32:64, :], dw_d[:])
            nc.sync.dma_start(c0b[:], c0b_d[:])
            lhsDr = [dwt[32:64, 16 * k : 16 * k + 16] for k in range(16)]
            lhsDs = [
                dwt[32:64, 256 + 16 * k : 256 + 16 * k + 16] for k in range(16)
            ]

            # stacked raw / s storage: step t -> [t % 128, t // 128, :]
            RAWa = store.tile([128, 4, BC], f32, tag="rawa")
            Sa = store.tile([128, 4, BC], f32, tag="sa")
            ACC = store.tile([128, 4], f32, tag="accs")

            # xr blocks: [4, XBLK*BC] bf16; rows: tau, 1, 1-m, m
            xrt = [None] * (L // XBLK)
            xrt[0] = xrp.tile([4, XBLK * BC], bf16, tag="xr", name="xr0")
            nc.sync.dma_start(xrt[0][:], xr_d[:, 0 : XBLK * BC])
            tqt = [None] * (L // 64)
            tqt[0] = tqp.tile([5, 16 * BC], bf16, tag="tq", name="tq0")
            nc.sync.dma_start(tqt[0][:], tq_d[:, 0 : 16 * BC])

            # h state (bf16); only consumed by negA
            hx = hx_pool.tile([32, BC], bf16, tag="hx")
            nc.vector.memset(hx[:], 0.0)

            gbank_cur = gP.tile([128, BC], f32, tag="gbank")
            nc.tensor.matmul(
                gbank_cur[:], W_x[:], xrt[0][0:3, 0:BC], start=True, stop=True
            )
            inP_cur = inPp.tile([128, BC], f32, tag="inP")
            nc.tensor.matmul(
                inP_cur[:], W_i[:], tqt[0][:, 0:BC], start=True, stop=True
            )

            dbank = None
            INs = [None] * L
            INs[0] = work.tile([64, BC], bf16, tag="INs", name="ins0")
            nc.vector.tensor_copy(INs[0][32:64, :], inP_cur[0:32, :])

            # Manual pacing: tile_set_cur_wait pins each instruction's
            # earliest simulated start, which fixes the scheduler's static
            # per-engine order (the HW then runs as fast as semaphores
            # allow, so generous pacing costs nothing at runtime).
            P = 0.012  # ms per step in the pacing model

            for t in range(L):
                gb = gbank_cur
                inb = inP_cur
                k = t % 16
                t0 = t * P

                # ---- critical path ----
                # one sigmoid for both gates: S = [zc; r]
                tc.tile_set_cur_wait(t0)
                S = work.tile([64, BC], bf16, tag="S")
                nc.scalar.activation(S[:], gb[0:64, :], AF.Sigmoid)

                if t < L - 1:
                    tc.tile_set_cur_wait(t0 + 0.0003)
                    if (t + 1) % XBLK == 0:
                        blk = (t + 1) // XBLK
                        xrt[blk] = xrp.tile(
                            [4, XBLK * BC], bf16, tag="xr", name=f"xr{blk}"
                        )
                        nc.sync.dma_start(
                            xrt[blk][:],
                            xr_d[:, XBLK * BC * blk : XBLK * BC * (blk + 1)],
                        )
                    xb_next = xrt[(t + 1) // XBLK]
                    xc = BC * ((t + 1) % XBLK)
                    gbank_next = gP.tile([128, BC], f32, tag="gbank")
                    nc.tensor.matmul(
                        gbank_next[:], W_x[:], xb_next[0:3, xc : xc + BC],
                        start=True, stop=False,
                    )

                # rh at partition 32 so all SBUF operands share a base
                tc.tile_set_cur_wait(t0 + 0.0019)
                RH = work.tile([64, BC], bf16, tag="RH")
                nc.vector.tensor_mul(RH[32:64, :], S[32:64, :], gb[64:96, :])
                # T2 in SBUF bf16 (2x mode); i_n slice was copied to
                # SBUF at partition 32 one step ahead
                tc.tile_set_cur_wait(t0 + 0.0033)
                T2t = work.tile([64, BC], bf16, tag="T2t")
                nc.vector.tensor_add(
                    T2t[32:64, :], RH[32:64, :], INs[t][32:64, :]
                )

                # i_n batch for the next 4-step group (prefetched)
                if t % 4 == 0 and t + 4 < L:
                    tc.tile_set_cur_wait(t0 + 0.0002)
                    g_next = t // 4 + 1
                    if g_next % 16 == 0:
                        tb = g_next // 16
                        tqt[tb] = tqp.tile(
                            [5, 16 * BC], bf16, tag="tq", name=f"tq{tb}"
                        )
                        nc.sync.dma_start(
                            tqt[tb][:], tq_d[:, 16 * BC * tb : 16 * BC * (tb + 1)]
                        )
                    tqb = tqt[g_next // 16]
                    tcol = BC * (g_next % 16)
                    inP_next = inPp.tile([128, BC], f32, tag="inP")
                    nc.tensor.matmul(
                        inP_next[:], W_i[:], tqb[:, tcol : tcol + BC],
                        start=True, stop=True,
                    )

                # n-tanh on the path (SBUF src); hazard tanh off-path
                tc.tile_set_cur_wait(t0 + 0.0046)
                NA = work.tile([64, BC], bf16, tag="NA")
                nc.scalar.activation(NA[0:32, :], T2t[32:64, :], AF.Tanh)

                tc.tile_set_cur_wait(t0 + 0.0060)
                negA = work.tile([32, BC], bf16, tag="negA")
                nc.vector.scalar_tensor_tensor(
                    negA[:], S[0:32, :], 1.0, hx[:], op0=ALU.subtract, op1=ALU.mult
                )
                tc.tile_set_cur_wait(t0 + 0.0062)
                nc.scalar.activation(NA[32:64, :], gb[96:128, :], AF.Tanh)
                if t < L - 1:
                    tc.tile_set_cur_wait(t0 + 0.0079)
                    nc.tensor.matmul(
                        gbank_next[:], W_un[:], negA[:], start=False, stop=False
                    )

                tc.tile_set_cur_wait(t0 + 0.0066)
                Bt = work.tile([32, BC], bf16, tag="Bt")
                nc.vector.tensor_mul(Bt[:], S[0:32, :], NA[0:32, :])
                if t < L - 1:
                    tc.tile_set_cur_wait(t0 + 0.0085)
                    nc.tensor.matmul(
                        gbank_next[:], W_u[:], Bt[:], start=False, stop=True
                    )
                    gbank_cur = gbank_next
                    if t % 4 == 3:
                        inP_cur = inP_next

                # ---- off-path ----
                if t < L - 1:
                    tc.tile_set_cur_wait(t0 + 0.0087)
                    hx_next = hx_pool.tile([32, BC], bf16, tag="hx")
                    nc.gpsimd.tensor_sub(hx_next[:], Bt[:], negA[:])
                    hx = hx_next
                tc.tile_set_cur_wait(t0 + 0.0095)
                SQ = work.tile([64, BC], bf16, tag="SQ")
                nc.gpsimd.tensor_mul(SQ[32:64, :], NA[32:64, :], NA[32:64, :])
                if t < L - 1:
                    tc.tile_set_cur_wait(t0 + 0.0092)
                    kq = (t + 1) % 4
                    ib = inP_cur if (t + 1) // 4 == t // 4 else inP_next
                    INs[t + 1] = work.tile(
                        [64, BC], bf16, tag="INs", name=f"ins{(t + 1) % 3}"
                    )
                    nc.vector.tensor_copy(
                        INs[t + 1][32:64, :], ib[32 * kq : 32 * kq + 32, :]
                    )

                # dots: raw -> dbank row k, s -> dbank row 32+k (16 steps)
                tc.tile_set_cur_wait(t0 + 0.0110)
                if k == 0:
                    dbank = dP.tile([48, BC], f32, tag="dbank")
                nc.tensor.matmul(
                    dbank[0:16, :], lhsDr[k], NA[32:64, :],
                    start=(k == 0), stop=(k == 15), tile_position=(32, 0),
                )
                nc.tensor.matmul(
                    dbank[32:48, :], lhsDs[k], SQ[32:64, :],
                    start=(k == 0), stop=(k == 15), tile_position=(32, 32),
                )

                if k == 15:
                    tc.tile_set_cur_wait(t0 + 0.0115)
                    g16 = t // 16
                    blk, row = g16 // 8, 16 * (g16 % 8)
                    stF = stg.tile([48, BC], f32, tag="stF", name=f"stF{g16 % 2}")
                    nc.vector.tensor_copy(stF[:], dbank[:])
                    nc.sync.dma_start(RAWa[row : row + 16, blk, :], stF[0:16, :])
                    nc.sync.dma_start(Sa[row : row + 16, blk, :], stF[32:48, :])

            # tail strictly after the loop in the schedule
            tc.tile_set_cur_wait(L * P + 0.01)

            # ---- batched loglik tail ----
            Mb, SG, ND, PH, SPD, LGL, LL, LLM = ([None] * 4 for _ in range(8))
            for i in range(4):
                Mb[i] = tailp.tile([128, BC], bf16, tag="Mb", name=f"Mb{i}")
                nc.sync.dma_start(Mb[i][:], mk_d[128 * i : 128 * (i + 1), :])
            for i in range(4):
                SG[i] = tailp.tile([128, BC], f32, tag="SG", name=f"SG{i}")
                nc.scalar.activation(
                    SG[i][:], RAWa[:, i, :], AF.Sigmoid, bias=c0b[:, 1:2]
                )
            for i in range(4):
                ND[i] = tailp.tile([128, BC], f32, tag="ND", name=f"ND{i}")
                nc.vector.scalar_tensor_tensor(
                    ND[i][:], Sa[:, i, :], c0b[:, 0:1], SG[i][:],
                    op0=ALU.subtract, op1=ALU.mult,
                )
            # softplus(x) = ln(1 + exp(x)) — no native softplus in this act
            # table set; exp and ln share a table. |raw|, |dphi| < ~8 so no
            # overflow.
            for i in range(4):
                EX = tailp.tile([128, BC], f32, tag="EX", name=f"EX{i}")
                nc.scalar.activation(EX[:], RAWa[:, i, :], AF.Exp, bias=c0b[:, 1:2])
                PH[i] = tailp.tile([128, BC], f32, tag="PH", name=f"PH{i}")
                nc.scalar.activation(PH[i][:], EX[:], AF.Ln, bias=1.0)
                EX2 = tailp.tile([128, BC], f32, tag="EX2", name=f"EX2{i}")
                nc.scalar.activation(EX2[:], ND[i][:], AF.Exp, scale=-1.0)
                SPD[i] = tailp.tile([128, BC], f32, tag="SPD", name=f"SPD{i}")
                nc.scalar.activation(SPD[i][:], EX2[:], AF.Ln, bias=1.0)
            for i in range(4):
                LGL[i] = tailp.tile([128, BC], f32, tag="LGL", name=f"LGL{i}")
                nc.scalar.activation(LGL[i][:], SPD[i][:], AF.Ln, bias=c0b[:, 2:3])
            for i in range(4):
                LL[i] = tailp.tile([128, BC], f32, tag="LL", name=f"LL{i}")
                nc.vector.tensor_sub(LL[i][:], LGL[i][:], PH[i][:])
                LLM[i] = tailp.tile([128, BC], f32, tag="LLM", name=f"LLM{i}")
                nc.vector.scalar_tensor_tensor(
                    LLM[i][:], LL[i][:], 0.0, Mb[i][:],
                    op0=ALU.add, op1=ALU.mult,
                    accum_out=ACC[:, i : i + 1],
                )
            nc.sync.dma_start(acc_d[:], ACC[:])

    nc.finalize()
    return nc


def _prep_host(inputs):
    d = {k: np.asarray(v, np.float32) for k, v in inputs.items()}
    w_ih, w_hh = d["w_ih"], d["w_hh"]
    b_ih, b_hh = d["b_ih"], d["b_hh"]
    w1, b1, w2, b2 = d["w1"], d["b1"], d["w2"], d["b2"]
    w1_tau, w1_h = w1[:, 0], w1[:, 1:]

    # W_x [3,128]: rows tau/1/(1-m) -> col blocks [zneg | r | h_n | pre]
    W_x = np.zeros((3, 128), np.float32)
    W_x[0, 0:32] = -w_ih[32:64, 0]
    W_x[1, 0:32] = -(b_ih[32:64] + b_hh[32:64])
    W_x[2, 0:32] = -BIG
    W_x[0, 32:64] = w_ih[0:32, 0]
    W_x[1, 32:64] = b_ih[0:32] + b_hh[0:32]
    W_x[1, 64:96] = b_hh[64:96]
    W_x[0, 96:128] = w1_tau
    W_x[1, 96:128] = b1

    # W_u [32,128]: recurrent parts (h enters via Bt - negA)
    W_u = np.zeros((32, 128), np.float32)
    W_u[:, 0:32] = -w_hh[32:64, :].T
    W_u[:, 32:64] = w_hh[0:32, :].T
    W_u[:, 64:96] = w_hh[64:96, :].T
    W_u[:, 96:128] = w1_h.T
    W_un = -W_u

    # W_i [5,128]: i_n for 4 steps; col block 32g+j: row g = w_ih_n[j],
    # row 4 = b_ih_n[j]
    W_i = np.zeros((5, 128), np.float32)
    for g in range(4):
        W_i[g, 32 * g : 32 * g + 32] = w_ih[64:96, 0]
        W_i[4, 32 * g : 32 * g + 32] = b_ih[64:96]

    # dot weight variants [32, 512]
    c = w1_tau * w2
    dw = np.zeros((32, 512), np.float32)
    for k in range(16):
        dw[:, 16 * k + k] = w2
        dw[:, 256 + 16 * k + k] = c

    c0b = np.tile(np.array([[c.sum(), b2[0], EPS]], np.float32), (128, 1))

    def to_bf16(x):
        import jax.numpy as jnp

        return np.asarray(jnp.asarray(x, jnp.bfloat16))

    deltas, mask = d["deltas"], d["mask"]
    wx16, wu16, wn16 = to_bf16(W_x), to_bf16(W_u), to_bf16(W_un)
    wi16, dw16 = to_bf16(W_i), to_bf16(dw)
    in_maps = []
    for i in range(NCORES):
        sl = slice(i * BC, (i + 1) * BC)
        dT = deltas[sl].T  # [L, BC]
        mT = mask[sl].T
        xr = np.empty((4, L * BC), np.float32)
        xr[0] = dT.reshape(-1)
        xr[1] = 1.0
        xr[2] = (1.0 - mT).reshape(-1)
        xr[3] = 0.0
        tq = np.empty((5, L // 4 * BC), np.float32)
        tq[0:4] = dT.reshape(L // 4, 4, BC).transpose(1, 0, 2).reshape(4, -1)
        tq[4] = 1.0
        in_maps.append(
            {
                "xr": to_bf16(xr),
                "tq": to_bf16(tq),
                "mk": to_bf16(mT),
                "wx": wx16,
                "wu": wu16,
                "wn": wn16,
                "wi": wi16,
                "dw": dw16,
                "c0b": c0b,
            }
        )
    return in_maps


def run_on_device(inputs, trace=False):
    from concourse.bass_utils import run_bass_kernel_spmd

    if "nc" not in _CACHE:
        _CACHE["nc"] = _build_module()
    nc = _CACHE["nc"]
    in_maps = _prep_host(inputs)
    res = run_bass_kernel_spmd(nc, in_maps, core_ids=list(range(NCORES)), trace=trace)
    tot = 0.0
    for r in res.results:
        tot += np.asarray(r["acc_out"], np.float64).sum()
    msum = np.asarray(inputs["mask"], np.float64).sum()
    out = np.float32(tot / (msum + EPS))
    return np.asarray(out, np.float32), res


def kernel(**inputs):
    out, _ = run_on_device(inputs, trace=False)
    return out
